# revision 1
# baseline (speedup 1.0000x reference)
"""TRN2 Bass kernel v3 for CrossOpLayerUTPM — batch-sharded, (i,d)-expanded.

out[b,(i,j)] = x[b,i] x[b,j] s[i,j].  Diagonals d=j-i in 8 chunks of 32.
Column layout m = OFFC[c] + i*32 + dd  (d = 1+32c+dd).

Per (chunk, batch-tile) two plain-2D bf16 DVE ops (all partition-base 0,
all contiguous free ranges — the only fast DVE paths on cayman):
    t1 = xE[:, 32*d0 : 32*(d0+W)] * xR[:, 0:32W]     # x[b,i+d] * x[b,i]
    o  = t1 * S[:, chunk]                             # * s[i,i+d]
xE[b, m*32+t] = x[b, m+t] (host-built sliding window), xR[b, i*32+dd] =
x[b, i] (host-built repeat), S broadcast rows (host).  bf16 out; host
drops garbage columns (j>255) and reorders pairs.
"""
import numpy as np
import ml_dtypes
from contextlib import ExitStack

import jax
from jax.sharding import Mesh, PartitionSpec
from jax.experimental.shard_map import shard_map

import concourse.bass as bass
import concourse.bacc as bacc
import concourse.tile as tile
from concourse import mybir
from concourse.bass2jax import (
    _bass_exec_p,
    install_neuronx_cc_hook,
    partition_id_tensor,
)

F32 = mybir.dt.float32
BF16 = mybir.dt.bfloat16
BF16NP = ml_dtypes.bfloat16

B, NCOL = 4096, 256
NCORES = 8
BPC = B // NCORES        # 512
NT = BPC // 128          # 4
NCH = 8
D0 = [1 + 32 * c for c in range(NCH)]
WC = [NCOL - d0 for d0 in D0]
OFFC = np.concatenate([[0], np.cumsum([32 * w for w in WC])]).astype(np.int64)
TOTF = int(OFFC[-1])     # 36608
XE_LEN = NCOL * 32       # 8192


def _build_nc(reps=1):
    nc = bacc.Bacc("TRN2", target_bir_lowering=False, debug=False)
    xe_in = nc.dram_tensor("xe", [BPC, XE_LEN], BF16, kind="ExternalInput")
    xr_in = nc.dram_tensor("xr", [BPC, XE_LEN], BF16, kind="ExternalInput")
    s_in = nc.dram_tensor("sb", [128, TOTF], BF16, kind="ExternalInput")
    out_t = nc.dram_tensor("out", [BPC, TOTF], BF16, kind="ExternalOutput")

    with tile.TileContext(nc) as tc, ExitStack() as ctx:
        cpool = ctx.enter_context(tc.tile_pool(name="const", bufs=1))
        xpool = ctx.enter_context(tc.tile_pool(name="xtiles", bufs=2))
        work = ctx.enter_context(tc.tile_pool(name="work", bufs=2))

        s_all = cpool.tile([128, TOTF], BF16, name="s_all")
        nc.sync.dma_start(out=s_all[:, :], in_=s_in[:, :])

        for r in range(reps):
          for t in range(NT):
            xe = xpool.tile([128, XE_LEN], BF16, tag="xe", name=f"xe{r}_{t}",
                            bufs=2)
            nc.sync.dma_start(out=xe[:, :], in_=xe_in[t * 128:(t + 1) * 128, :])
            xr = xpool.tile([128, XE_LEN], BF16, tag="xr", name=f"xr{r}_{t}",
                            bufs=2)
            nc.sync.dma_start(out=xr[:, :], in_=xr_in[t * 128:(t + 1) * 128, :])
            for c in range(NCH):
                d0, w = D0[c], WC[c]
                for h in range(2):
                    wlo = (w // 2) * h
                    whi = w if h else (w // 2)
                    fsz = 32 * (whi - wlo)
                    off = int(OFFC[c]) + 32 * wlo
                    exo = 32 * (d0 + wlo)
                    t1 = work.tile([128, 4096], BF16, tag="t1",
                                   name=f"t{r}_{c}_{t}_{h}", bufs=3)
                    nc.vector.tensor_mul(
                        t1[:, 0:fsz], xe[:, exo:exo + fsz],
                        xr[:, 32 * wlo:32 * wlo + fsz])
                    o = work.tile([128, 4096], BF16, tag="o",
                                  name=f"o{r}_{c}_{t}_{h}", bufs=3)
                    nc.vector.tensor_mul(o[:, 0:fsz], t1[:, 0:fsz],
                                         s_all[:, off:off + fsz])
                    nc.sync.dma_start(
                        out=out_t[t * 128:(t + 1) * 128, off:off + fsz],
                        in_=o[:, 0:fsz])

    nc.compile()
    return nc


class _Runner:
    def __init__(self, nc, n_cores=NCORES):
        install_neuronx_cc_hook()
        self.nc = nc
        self.n_cores = n_cores
        partition_name = (
            nc.partition_id_tensor.name if nc.partition_id_tensor else None
        )
        in_names, out_names, out_avals, zero_outs = [], [], [], []
        for alloc in nc.m.functions[0].allocations:
            if not isinstance(alloc, mybir.MemoryLocationSet):
                continue
            name = alloc.memorylocations[0].name
            if alloc.kind == "ExternalInput":
                if name != partition_name:
                    in_names.append(name)
            elif alloc.kind == "ExternalOutput":
                shape = tuple(alloc.tensor_shape)
                dtype = mybir.dt.np(alloc.dtype)
                out_avals.append(jax.core.ShapedArray(shape, dtype))
                zero_outs.append(np.zeros(shape, dtype))
                out_names.append(name)
        self.n_params = len(in_names)
        self.param_names = list(in_names)
        self.out_names = out_names
        self.out_avals = out_avals
        self.zero_outs = zero_outs
        all_in = in_names + out_names
        if partition_name is not None:
            all_in.append(partition_name)

        def _body(*args):
            operands = list(args)
            if partition_name is not None:
                operands.append(partition_id_tensor())
            return tuple(_bass_exec_p.bind(
                *operands,
                out_avals=tuple(out_avals),
                in_names=tuple(all_in),
                out_names=tuple(out_names),
                lowering_input_output_aliases=(),
                sim_require_finite=False,
                sim_require_nnan=False,
                nc=nc,
            ))

        devices = jax.devices()[:n_cores]
        mesh = Mesh(np.asarray(devices), ("core",))
        n_outs = len(out_names)
        in_specs = (PartitionSpec("core"),) * (self.n_params + n_outs)
        out_specs = (PartitionSpec("core"),) * n_outs
        self.fn = jax.jit(
            shard_map(_body, mesh=mesh, in_specs=in_specs,
                      out_specs=out_specs, check_rep=False),
            keep_unused=True,
        )

    def run_concat(self, concat_in):
        concat_zeros = [
            np.zeros((self.n_cores * z.shape[0], *z.shape[1:]), z.dtype)
            for z in self.zero_outs
        ]
        outs = self.fn(*concat_in, *concat_zeros)
        return [np.asarray(o) for o in outs]


_CACHE = {}


def _get_runner(reps=1):
    if reps not in _CACHE:
        _CACHE[reps] = _Runner(_build_nc(reps))
    return _CACHE[reps]


def _host_prep(x, latent_emb):
    x = np.asarray(x, np.float32)
    L = np.asarray(latent_emb, np.float32)
    s = (L @ L.T).astype(np.float32)

    # S_flat[(c, i, dd)] = s[i, i + 1 + 32c + dd] (0 where j > 255)
    s_flat = np.zeros(TOTF, np.float32)
    for c in range(NCH):
        d0, w = D0[c], WC[c]
        ii, dd = np.meshgrid(np.arange(w), np.arange(32), indexing="ij")
        j = ii + d0 + dd
        blk = np.zeros((w, 32), np.float32)
        valid = j <= NCOL - 1
        blk[valid] = s[ii[valid], j[valid]]
        s_flat[OFFC[c]:OFFC[c + 1]] = blk.reshape(-1)
    s_bcast = np.broadcast_to(s_flat.astype(BF16NP), (128, TOTF)).copy()

    # per-core xE (sliding windows) and xR (32x repeat)
    xb = x.astype(BF16NP)
    xpad = np.zeros((B, NCOL + 32), BF16NP)
    xpad[:, :NCOL] = xb
    win = np.lib.stride_tricks.sliding_window_view(
        xpad, 32, axis=1)[:, :NCOL, :]                    # [B, 256, 32]
    xE = win.reshape(B, XE_LEN)
    xR = np.repeat(xb, 32, axis=1)                        # [B, 8192]

    xe_cores = [np.ascontiguousarray(xE[c * BPC:(c + 1) * BPC])
                for c in range(NCORES)]
    xr_cores = [np.ascontiguousarray(xR[c * BPC:(c + 1) * BPC])
                for c in range(NCORES)]
    return xe_cores, xr_cores, s_bcast


_IDX = None


def _pair_index():
    global _IDX
    if _IDX is None:
        iu, ju = np.triu_indices(NCOL, k=1)
        d = ju - iu
        c = (d - 1) // 32
        dd = d - 1 - 32 * c
        _IDX = (OFFC[c] + iu * 32 + dd).astype(np.int64)
    return _IDX


def kernel(x, latent_emb):
    xe_cores, xr_cores, s_bcast = _host_prep(x, latent_emb)
    runner = _get_runner()
    concat_in = []
    for name in runner.param_names:
        if name == "xe":
            concat_in.append(np.concatenate(xe_cores, axis=0))
        elif name == "xr":
            concat_in.append(np.concatenate(xr_cores, axis=0))
        elif name == "sb":
            concat_in.append(np.concatenate([s_bcast] * NCORES, axis=0))
        else:
            raise KeyError(name)
    outs = runner.run_concat(concat_in)
    dev = outs[runner.out_names.index("out")]     # [4096, TOTF] bf16
    return dev[:, _pair_index()].astype(np.float32)



# revision 2
# speedup vs baseline: 1.1582x; 1.1582x over previous
"""TRN2 Bass kernel v7 for CrossOpLayerUTPM — batch-sharded, window-AP xE.

out[b,(i,j)] = x[b,i] x[b,j] s[i,j].  Diagonals d=j-i in 8 chunks of 32.
Column layout m = OFFC[c] + i*32 + dd  (d = 1+32c+dd).

Per (chunk, batch-tile) two plain bf16 DVE tensor_tensor ops (2x mode):
    t1 = window(xp)[i', dd] * xR[:, ...]   # x[b,i'+dd] * x[b,i']
    o  = t1 * S[:, chunk]                  # * s[i, i+d]
The sliding-window operand is read straight from a resident padded x
tile via an overlapping 3D access pattern ([p][win: stride 1 elem][32]),
eliminating the 8.4 MB/core xE DMA stream of the previous version
(HW-verified: this AP shape keeps the DVE bf16 2x tensor_tensor mode;
a stride-0 repeat AP does not, so xR stays host-built).  bf16 out; host
drops garbage columns (j>255) and reorders pairs.
"""
import numpy as np
import ml_dtypes
from contextlib import ExitStack

import jax
from jax.sharding import Mesh, PartitionSpec
from jax.experimental.shard_map import shard_map

import bass_rust
import concourse.bass as bass  # noqa: F401
import concourse.bacc as bacc
import concourse.tile as tile
from concourse import mybir
from concourse.bass2jax import (
    _bass_exec_p,
    install_neuronx_cc_hook,
    partition_id_tensor,
)

F32 = mybir.dt.float32
BF16 = mybir.dt.bfloat16
BF16NP = ml_dtypes.bfloat16

B, NCOL = 4096, 256
NCORES = 8
BPC = B // NCORES        # 512
NT = BPC // 128          # 4
NCH = 8
D0 = [1 + 32 * c for c in range(NCH)]
WC = [NCOL - d0 for d0 in D0]
OFFC = np.concatenate([[0], np.cumsum([32 * w for w in WC])]).astype(np.int64)
TOTF = int(OFFC[-1])     # 36608
XE_LEN = NCOL * 32       # 8192
XPAD = NCOL + 32         # 288


def _win_ap(xp, base, nwin):
    """AP over xp reading windows x[base+i'+dd], i'<nwin, dd<32."""
    w = xp[:, base:base + 1].copy()
    w.ap = bass_rust.VecI64Pair([tuple(w.ap[0]), (1, nwin), (1, 32)])
    return w


def _build_nc(reps=1):
    nc = bacc.Bacc("TRN2", target_bir_lowering=False, debug=False)
    xp_in = nc.dram_tensor("xp", [BPC, XPAD], BF16, kind="ExternalInput")
    xr_in = nc.dram_tensor("xr", [BPC, XE_LEN], BF16, kind="ExternalInput")
    s_in = nc.dram_tensor("sb", [128, TOTF], BF16, kind="ExternalInput")
    out_t = nc.dram_tensor("out", [BPC, TOTF], BF16, kind="ExternalOutput")

    with tile.TileContext(nc) as tc, ExitStack() as ctx:
        cpool = ctx.enter_context(tc.tile_pool(name="const", bufs=1))
        xpool = ctx.enter_context(tc.tile_pool(name="xtiles", bufs=2))
        work = ctx.enter_context(tc.tile_pool(name="work", bufs=2))

        s_all = cpool.tile([128, TOTF], BF16, name="s_all")
        nc.sync.dma_start(out=s_all[:, :], in_=s_in[:, :])

        for r in range(reps):
          for t in range(NT):
            xp = xpool.tile([128, XPAD], BF16, tag="xp", name=f"xp{r}_{t}",
                            bufs=3)
            nc.sync.dma_start(out=xp[:, :], in_=xp_in[t * 128:(t + 1) * 128, :])
            xr = xpool.tile([128, XE_LEN], BF16, tag="xr", name=f"xr{r}_{t}",
                            bufs=2)
            nc.sync.dma_start(out=xr[:, :], in_=xr_in[t * 128:(t + 1) * 128, :])
            for c in range(NCH):
                d0, w = D0[c], WC[c]
                for h in range(2):
                    wlo = (w // 2) * h
                    whi = w if h else (w // 2)
                    nwin = whi - wlo
                    fsz = 32 * nwin
                    off = int(OFFC[c]) + 32 * wlo
                    t1 = work.tile([128, 4096], BF16, tag="t1",
                                   name=f"t{r}_{c}_{t}_{h}", bufs=3)
                    nc.vector.tensor_mul(
                        t1[:, 0:fsz], _win_ap(xp, d0 + wlo, nwin),
                        xr[:, 32 * wlo:32 * wlo + fsz])
                    o = work.tile([128, 4096], BF16, tag="o",
                                  name=f"o{r}_{c}_{t}_{h}", bufs=3)
                    nc.vector.tensor_mul(o[:, 0:fsz], t1[:, 0:fsz],
                                         s_all[:, off:off + fsz])
                    nc.sync.dma_start(
                        out=out_t[t * 128:(t + 1) * 128, off:off + fsz],
                        in_=o[:, 0:fsz])

    nc.compile()
    return nc


class _Runner:
    def __init__(self, nc, n_cores=NCORES):
        install_neuronx_cc_hook()
        self.nc = nc
        self.n_cores = n_cores
        partition_name = (
            nc.partition_id_tensor.name if nc.partition_id_tensor else None
        )
        in_names, out_names, out_avals, zero_outs = [], [], [], []
        for alloc in nc.m.functions[0].allocations:
            if not isinstance(alloc, mybir.MemoryLocationSet):
                continue
            name = alloc.memorylocations[0].name
            if alloc.kind == "ExternalInput":
                if name != partition_name:
                    in_names.append(name)
            elif alloc.kind == "ExternalOutput":
                shape = tuple(alloc.tensor_shape)
                dtype = mybir.dt.np(alloc.dtype)
                out_avals.append(jax.core.ShapedArray(shape, dtype))
                zero_outs.append(np.zeros(shape, dtype))
                out_names.append(name)
        self.n_params = len(in_names)
        self.param_names = list(in_names)
        self.out_names = out_names
        self.out_avals = out_avals
        self.zero_outs = zero_outs
        all_in = in_names + out_names
        if partition_name is not None:
            all_in.append(partition_name)

        def _body(*args):
            operands = list(args)
            if partition_name is not None:
                operands.append(partition_id_tensor())
            return tuple(_bass_exec_p.bind(
                *operands,
                out_avals=tuple(out_avals),
                in_names=tuple(all_in),
                out_names=tuple(out_names),
                lowering_input_output_aliases=(),
                sim_require_finite=False,
                sim_require_nnan=False,
                nc=nc,
            ))

        devices = jax.devices()[:n_cores]
        mesh = Mesh(np.asarray(devices), ("core",))
        n_outs = len(out_names)
        in_specs = (PartitionSpec("core"),) * (self.n_params + n_outs)
        out_specs = (PartitionSpec("core"),) * n_outs
        self.fn = jax.jit(
            shard_map(_body, mesh=mesh, in_specs=in_specs,
                      out_specs=out_specs, check_rep=False),
            keep_unused=True,
        )

    def run_concat(self, concat_in):
        concat_zeros = [
            np.zeros((self.n_cores * z.shape[0], *z.shape[1:]), z.dtype)
            for z in self.zero_outs
        ]
        outs = self.fn(*concat_in, *concat_zeros)
        return [np.asarray(o) for o in outs]


_CACHE = {}


def _get_runner(reps=1):
    if reps not in _CACHE:
        _CACHE[reps] = _Runner(_build_nc(reps))
    return _CACHE[reps]


def _host_prep(x, latent_emb):
    x = np.asarray(x, np.float32)
    L = np.asarray(latent_emb, np.float32)
    s = (L @ L.T).astype(np.float32)

    # S_flat[(c, i, dd)] = s[i, i + 1 + 32c + dd] (0 where j > 255)
    s_flat = np.zeros(TOTF, np.float32)
    for c in range(NCH):
        d0, w = D0[c], WC[c]
        ii, dd = np.meshgrid(np.arange(w), np.arange(32), indexing="ij")
        j = ii + d0 + dd
        blk = np.zeros((w, 32), np.float32)
        valid = j <= NCOL - 1
        blk[valid] = s[ii[valid], j[valid]]
        s_flat[OFFC[c]:OFFC[c + 1]] = blk.reshape(-1)
    s_bcast = np.broadcast_to(s_flat.astype(BF16NP), (128, TOTF)).copy()

    xb = x.astype(BF16NP)
    xpad = np.zeros((B, XPAD), BF16NP)
    xpad[:, :NCOL] = xb
    xR = np.repeat(xb, 32, axis=1)                        # [B, 8192]

    xp_cores = [np.ascontiguousarray(xpad[c * BPC:(c + 1) * BPC])
                for c in range(NCORES)]
    xr_cores = [np.ascontiguousarray(xR[c * BPC:(c + 1) * BPC])
                for c in range(NCORES)]
    return xp_cores, xr_cores, s_bcast


_IDX = None


def _pair_index():
    global _IDX
    if _IDX is None:
        iu, ju = np.triu_indices(NCOL, k=1)
        d = ju - iu
        c = (d - 1) // 32
        dd = d - 1 - 32 * c
        _IDX = (OFFC[c] + iu * 32 + dd).astype(np.int64)
    return _IDX


def kernel(x, latent_emb):
    xp_cores, xr_cores, s_bcast = _host_prep(x, latent_emb)
    runner = _get_runner()
    concat_in = []
    for name in runner.param_names:
        if name == "xp":
            concat_in.append(np.concatenate(xp_cores, axis=0))
        elif name == "xr":
            concat_in.append(np.concatenate(xr_cores, axis=0))
        elif name == "sb":
            concat_in.append(np.concatenate([s_bcast] * NCORES, axis=0))
        else:
            raise KeyError(name)
    outs = runner.run_concat(concat_in)
    dev = outs[runner.out_names.index("out")]     # [4096, TOTF] bf16
    return dev[:, _pair_index()].astype(np.float32)
